# revision 36
# baseline (speedup 1.0000x reference)
"""Canny edge detection (nn_CannyEdge_83330955477492) on 8 Trainium2 cores.

Pipeline reproduced from the reference:
  - The reference's "gaussian blur" (sigma=0.05, and a 2x2 kernel thanks to
    arange(-(3//2)+1, 3//2+1) == [0,1]) is exactly a top-left crop of the
    reflect-padded image: blur[i,j] = x[R(i-1), R(j-1)], R(-1)=1 -> 1025x1025.
  - Sobel gx/gy on the reflect-padded blur (correlation).
  - Direction binning by exact slope comparisons on the SQUARES
    (T^2*gx^2 vs gy^2) instead of atan2 (bit-identical except for pixels
    within ~1 ulp of a bin boundary).
  - Magnitude comparisons use gx^2+gy^2 (monotone equivalent of sqrt).
  - NMS via shifted maxes per bin; thresholds at 50^2/100^2.

Sharding: pure data parallel, 2 images per core.

Layout: 128 partitions (64 per image), 16 output rows per partition,
9 column chunks of 116.  The device emits five u8 indicator maps (4 per-bin
weak-edge bits + the >=100 bit) packed in one contiguous store per chunk;
the host sums/crops them into the three 0/255 float maps.  Image rows 0 and
1024 (which see the NMS zero-padding) are recomputed exactly on the host,
which removes all border masking from the device inner loop.

Work is split across three engines, restricted to what the real walrus
backend accepts per engine (Pool = gpsimd: tensor_tensor add/sub/mult and
tensor_scalar only, SBUF only; ACT: 1-input affine+func; DVE: everything):
  Pool: all adds/subs/mults (tt, rsm/gx assembly, dd, t2, gy, sg, mm, the
        masked-magnitude builds) + two bins' weak-bit chains via the exact
        sign-of-subtract trick  w = [cen - max(qt,50^2) >= 0].
  ACT:  the 2*center scaled copies and the gradient squares.
  DVE:  all compares (bin masks, NMS maxes, weak/sure bits).
All arithmetic is plain fp32, bit-identical to the reference up to ulp-level
comparison ties (the fp32r tensor-engine path was rejected: real-HW fp32r
rounds its operands, flipping ~5k near-tie NMS comparisons).
"""
import numpy as np

# ---------------------------------------------------------------- geometry
NIMG = 2             # images per core
H = 1024             # input image size
HO = 1025            # output size (blur is 1025x1025)
RPP = 16             # output rows per partition
PPI = 64             # partitions per image
NPART = NIMG * PPI   # 128
CW = 116             # output cols per chunk
NCH = 9              # chunks (9*116 = 1044 >= 1025)
WA = CW + 4          # loaded cols per chunk (even: fp32r matmul spans)
WB = CW + 2          # stencil cols per chunk (even)
TROWS = 21           # tin rows (20 + 1 pad so shifted flat spans stay in)
QROWS_IMG = 1029     # P's 1027 rows + 2 zero pad
QCOLS = NCH * CW + WA - CW  # 1 zero col + P's 1027 cols + pad
LASTC = HO - CW * (NCH - 1) + 1  # mm col of blur col 1025 in last chunk

_T1 = np.float32(np.tan(np.deg2rad(22.5)))
_T2 = np.float32(np.tan(np.deg2rad(67.5)))
T1S = float(np.float32(_T1 * _T1))
T2S = float(np.float32(_T2 * _T2))
MIN2 = float(np.float32(50.0 * 50.0))
MAX2 = float(np.float32(100.0 * 100.0))

_NC = None
LAST_RESULTS = None  # stashed BassKernelResults for test.py


# ------------------------------------------------- walrus 1-wait workaround
def _split_multiwaits(nc):
    """This walrus build rejects >1 sync wait per instruction: move extra
    waits onto fresh same-engine NOPs inserted right before the carrier."""
    import concourse.mybir as mybir

    n_split = 0
    for fn in nc.m.functions:
        for bb in fn.blocks:
            insts = list(bb.instructions)
            if not any(
                i.sync_info is not None
                and i.sync_info.on_wait
                and len(i.sync_info.on_wait) > 1
                for i in insts
            ):
                continue
            out = []
            for inst in insts:
                si = inst.sync_info
                if si is not None and si.on_wait and len(si.on_wait) > 1:
                    waits = list(si.on_wait)
                    eng = nc.engines[inst.engine]
                    for w in waits[:-1]:
                        nop = eng.nop(hint="waitsplit")
                        # eng.nop() appended to nc.cur_bb — remove it there
                        # (it must live ONLY at its split position, else the
                        # duplicate runs after sem cleanup and deadlocks).
                        host = nc.cur_bb.bb
                        lst = list(host.instructions)
                        assert lst and lst[-1].name == nop.ins.name
                        _set_insts(host, lst[:-1])
                        nop.ins.sync_info = mybir.SyncInfo(
                            on_wait=[w], on_update=[]
                        )
                        out.append(nop.ins)
                        n_split += 1
                    si.on_wait = waits[-1:]
                out.append(inst)
            _set_insts(bb, out)
    return n_split


def _set_insts(bb, lst):
    try:
        bb.instructions = lst
    except Exception:
        bb.instructions.clear()
        bb.instructions.extend(lst)


# ------------------------------------------------------------ device build
#
# v3: the PE (tensor engine) computes every pure add/sub of (shifted) views
# as float32r identity matmuls accumulated in PSUM (bit-exact: products are
# 1.0*x or 2.0*x with exact fp32 PSUM accumulation).  Each matmul output is
# one PSUM bank (3 rows x WB = 393 f32 <= 512); tensors are processed in two
# row-halves so two 4-bank PSUM slots ping-pong.  ACT drains PSUM into dense
# SBUF tiles (fused with Square for gx/gy); DVE/Pool keep the tensor
# multiplies and all NMS compares.
def _build_nc():
    import concourse.bass as bass
    import concourse.tile as tile
    import concourse.mybir as mybir
    from concourse.ap import AP

    f32 = mybir.dt.float32
    f32r = mybir.dt.float32r
    u8 = mybir.dt.uint8
    Alu = mybir.AluOpType
    Act = mybir.ActivationFunctionType

    nc = bass.Bass("TRN2", target_bir_lowering=False, debug=False, num_devices=8)
    qc = nc.declare_dram_parameter("qc", [NCH, NPART, TROWS, WA], f32,
                                   isOutput=False)
    ou = nc.declare_dram_parameter("ou", [NCH, NPART, 5, RPP, CW], u8,
                                   isOutput=True)

    with tile.TileContext(nc) as tc:
        with (
            tc.tile_pool(name="io2", bufs=2) as io2,   # load/store overlap
            tc.tile_pool(name="mid", bufs=1) as mid,   # per-chunk work tiles
        ):
            def stage1(ci):
                """Load + row stencils for chunk ci."""
                tin = io2.tile([NPART, TROWS, WA], f32, tag="tin")
                src = AP(qc, ci * NPART * TROWS * WA,
                         [[TROWS * WA, NPART], [WA, TROWS], [1, WA]])
                nc.sync.dma_start(out=tin[:], in_=src)

                # rsm = [1,2,1] cols = 2C + (L+R);  dd = R - L
                tt = mid.tile([NPART, 20, WB], f32, tag="tt")
                nc.gpsimd.tensor_tensor(
                    out=tt[:], in0=tin[:, 0:20, 0:WB], in1=tin[:, 0:20, 2:WA],
                    op=Alu.add)
                u2 = mid.tile([NPART, 20, WB], f32, tag="u2")
                nc.scalar.activation(out=u2[:], in_=tin[:, 0:20, 1:WB + 1],
                                     func=Act.Copy, scale=2.0)
                rsm = mid.tile([NPART, 20, WB], f32, tag="rsm", bufs=2)
                nc.gpsimd.tensor_tensor(out=rsm[:], in0=u2[:], in1=tt[:],
                                        op=Alu.add)
                dd = mid.tile([NPART, 20, WB], f32, tag="dd", bufs=2)
                nc.gpsimd.tensor_tensor(
                    out=dd[:], in0=tin[:, 0:20, 2:WA], in1=tin[:, 0:20, 0:WB],
                    op=Alu.subtract)
                return dict(rsm=rsm, dd=dd)

            def stage2(ci, st):
                """Gradients, sign, squares, bin masks, magnitude."""
                rsm, dd = st["rsm"], st["dd"]
                # gx = [1,2,1] rows of dd = 2*ddC + (ddU + ddD)
                t2 = mid.tile([NPART, 18, WB], f32, tag="t2")
                nc.gpsimd.tensor_tensor(
                    out=t2[:], in0=dd[:, 0:18, :], in1=dd[:, 2:20, :],
                    op=Alu.add)
                u3 = mid.tile([NPART, 18, WB], f32, tag="u2")
                nc.scalar.activation(out=u3[:], in_=dd[:, 1:19, :],
                                     func=Act.Copy, scale=2.0)
                gx = mid.tile([NPART, 18, WB], f32, tag="gx")
                nc.gpsimd.tensor_tensor(out=gx[:], in0=u3[:], in1=t2[:],
                                        op=Alu.add)
                gy = mid.tile([NPART, 18, WB], f32, tag="gy")
                nc.vector.tensor_tensor(
                    out=gy[:], in0=rsm[:, 2:20, :], in1=rsm[:, 0:18, :],
                    op=Alu.subtract)
                sg = mid.tile([NPART, 18, WB], f32, tag="sg")
                nc.gpsimd.tensor_tensor(out=sg[:], in0=gx[:], in1=gy[:],
                                        op=Alu.mult)
                # squares in place (ACT)
                nc.scalar.activation(out=gx[:], in_=gx[:], func=Act.Square)
                nc.scalar.activation(out=gy[:], in_=gy[:], func=Act.Square)

                # bin masks from the squares (DVE)
                c0 = mid.tile([NPART, 18, WB], f32, tag="c0")
                nc.vector.scalar_tensor_tensor(
                    out=c0[:], in0=gx[:], scalar=T1S, in1=gy[:],
                    op0=Alu.mult, op1=Alu.is_ge)
                d2 = mid.tile([NPART, 18, WB], f32, tag="d2")
                nc.vector.scalar_tensor_tensor(
                    out=d2[:], in0=gx[:], scalar=T2S, in1=gy[:],
                    op0=Alu.mult, op1=Alu.is_gt)

                # magnitude^2 + NMS zero-pad cols at image edges
                mm = mid.tile([NPART, 18, WB], f32, tag="mm")
                nc.gpsimd.tensor_tensor(out=mm[:], in0=gx[:], in1=gy[:],
                                        op=Alu.add)
                if ci == 0:
                    nc.gpsimd.memset(mm[:, :, 0:1], 0.0)       # blur col -1
                if ci == NCH - 1:
                    nc.gpsimd.memset(mm[:, :, LASTC:LASTC + 1], 0.0)
                return dict(sg=sg, c0=c0, d2=d2, mm=mm)

            def stage3(ci, st):
                """NMS per-bin processing + store for chunk ci."""
                sg, c0, d2, mm = st["sg"], st["c0"], st["d2"], st["mm"]
                ws = io2.tile([NPART, 5, RPP, CW], u8, tag="ws")

                def wslot(s):
                    return ws[:, s:s + 1].rearrange("p a r c -> p (a r) c")

                nc.vector.tensor_scalar(
                    out=wslot(4), in0=mm[:, 1:17, 1:1 + CW], scalar1=MAX2,
                    scalar2=None, op0=Alu.is_ge)

                def nms_bin(ang, r1, c1, r2, c2, slot, pool_w):
                    # qt = max of the two in-bin neighbours (DVE: Pool has
                    # no tensor max).  pool_w offloads the weak-bit to Pool
                    # as [cen - max(qt, MIN2) >= 0] (sign of an IEEE
                    # subtract is exact).
                    qt = mid.tile([NPART, RPP, CW], f32, tag="qt", bufs=2)
                    nc.vector.tensor_tensor(
                        out=qt[:],
                        in0=ang[:, r1:r1 + RPP, c1:c1 + CW],
                        in1=ang[:, r2:r2 + RPP, c2:c2 + CW],
                        op=Alu.max)
                    cen = ang[:, 1:17, 1:1 + CW]
                    if not pool_w:
                        nc.vector.scalar_tensor_tensor(
                            out=wslot(slot), in0=qt[:], scalar=MIN2,
                            in1=cen, op0=Alu.max, op1=Alu.is_le)
                    else:
                        nc.gpsimd.tensor_scalar(
                            out=qt[:], in0=qt[:], scalar1=MIN2,
                            scalar2=None, op0=Alu.max)
                        nc.gpsimd.tensor_tensor(
                            out=qt[:], in0=cen, in1=qt[:], op=Alu.subtract)
                        nc.gpsimd.tensor_scalar(
                            out=wslot(slot), in0=qt[:], scalar1=0.0,
                            scalar2=None, op0=Alu.is_ge)

                # bins are disjoint; each w bit is 1 only where the center
                # is in the bin and >= max(neighbors, 50^2).
                md2 = mid.tile([NPART, 18, WB], f32, tag="md2")
                nc.gpsimd.tensor_tensor(out=md2[:], in0=mm[:], in1=d2[:],
                                        op=Alu.mult)      # Md2 = M*d2
                angA = mid.tile([NPART, 18, WB], f32, tag="angA")
                nc.gpsimd.tensor_tensor(out=angA[:], in0=mm[:], in1=md2[:],
                                        op=Alu.subtract)  # ang2 = M - Md2
                nms_bin(angA, 0, 1, 2, 1, 2, False)       # bin2: up/down
                angB = mid.tile([NPART, 18, WB], f32, tag="angB")
                nc.gpsimd.tensor_tensor(out=angB[:], in0=md2[:], in1=c0[:],
                                        op=Alu.mult)      # ang0 = Md2*c0
                nms_bin(angB, 1, 0, 1, 2, 0, True)        # bin0: left/right
                mdg = mid.tile([NPART, 18, WB], f32, tag="angA")
                nc.gpsimd.tensor_tensor(out=mdg[:], in0=md2[:], in1=angB[:],
                                        op=Alu.subtract)  # mdiag
                angC = mid.tile([NPART, 18, WB], f32, tag="angC")
                nc.vector.scalar_tensor_tensor(
                    out=angC[:], in0=sg[:], scalar=0.0, in1=mdg[:],
                    op0=Alu.is_gt, op1=Alu.mult)          # ang3 = (s>0)*mdiag
                nms_bin(angC, 0, 0, 2, 2, 3, False)       # bin3: main diag
                ang1 = mid.tile([NPART, 18, WB], f32, tag="angB")
                nc.gpsimd.tensor_tensor(out=ang1[:], in0=mdg[:],
                                        in1=angC[:], op=Alu.subtract)
                nms_bin(ang1, 0, 2, 2, 0, 1, True)        # bin1: anti diag

                dst = AP(ou, ci * NPART * 5 * RPP * CW,
                         [[5 * RPP * CW, NPART], [RPP * CW, 5],
                          [CW, RPP], [1, CW]])
                nc.sync.dma_start(out=dst, in_=ws[:])

            # 2-deep software pipeline: stage1(ci) issues before
            # stages 2+3 of chunk ci-1
            s1 = {}
            for ci in range(NCH + 1):
                if ci < NCH:
                    s1[ci] = stage1(ci)
                if ci >= 1:
                    stage3(ci - 1, stage2(ci - 1, s1.pop(ci - 1)))

    _split_multiwaits(nc)
    return nc


def _get_nc():
    global _NC
    if _NC is None:
        _NC = _build_nc()
    return _NC


# ------------------------------------------------------------- host helpers
def _eye3():
    """Stationary matrices for the PE identity matmuls: I, 2I, -I."""
    e = np.eye(NPART, dtype=np.float32)
    return np.stack([e, 2.0 * e, -e], axis=1).astype(np.float32)


def _pad_idx():
    idx = np.empty(1027, np.int64)
    idx[0] = 0
    idx[1] = 1
    idx[2:1026] = np.arange(1024)
    idx[1026] = 1022
    return idx


def _build_qc(images):
    """images: (16, 1024, 1024) f32 -> per-core chunked input
    (8, NCH, NPART, 20, WA) with all halos baked in so every load is one
    contiguous descriptor per partition."""
    idx = _pad_idx()
    qcs = np.zeros((8, NCH, NPART, TROWS, WA), np.float32)
    win = np.zeros((NPART, TROWS, QCOLS), np.float32)
    for core in range(8):
        for k in range(NIMG):
            im = images[core * NIMG + k]
            p = im[idx][:, idx]                      # (1027, 1027) = bp rows
            q = np.zeros((QROWS_IMG, QCOLS), np.float32)
            q[0:1027, 1:1028] = p
            # partition p' takes rows 16p'..16p'+20
            sw = np.lib.stride_tricks.sliding_window_view(q, TROWS, axis=0)
            win[k * PPI:(k + 1) * PPI] = sw[0:16 * PPI:16].transpose(0, 2, 1)
        for ci in range(NCH):
            qcs[core, ci] = win[:, :, CW * ci:CW * ci + WA]
    return qcs


def _strip_rows(p):
    """Exact f32 Canny decision bits for output rows 0 and 1024.

    p: (1027, 1027) f32 padded blur (rows/cols -1..1025).
    Returns (w50, big) as uint8 arrays of shape (2, 1025)."""
    f = np.float32
    w50 = np.zeros((2, HO), np.uint8)
    big = np.zeros((2, HO), np.uint8)
    for oi, r in enumerate((0, H)):
        rows = [j for j in (r - 1, r, r + 1) if 0 <= j <= H]
        mm = {}
        a2 = {}
        a0 = {}
        a3 = {}
        a1 = {}
        for j in rows:
            s = p[j:j + 3]                           # 3 x 1027
            ddr = s[:, 2:] - s[:, :-2]               # 3 x 1025
            ssr = s[:, :-2] + f(2.0) * s[:, 1:-1] + s[:, 2:]
            gx = ddr[0] + f(2.0) * ddr[1] + ddr[2]
            gy = ssr[2] - ssr[0]
            m = gx * gx + gy * gy
            ax = np.abs(gx)
            ay = np.abs(gy)
            c0 = (_T1 * ax >= ay)
            d2 = (_T2 * ax > ay)
            sgp = (gx * gy) > 0
            mm[j] = m
            a2[j] = np.where(d2, f(0), m)
            md = np.where(d2, m, f(0))
            a0[j] = np.where(c0, md, f(0))
            mdg = md - a0[j]
            a3[j] = np.where(sgp, mdg, f(0))
            a1[j] = mdg - a3[j]
        z = np.zeros(HO, np.float32)

        def sh(v, d):  # shift cols by d with zero pad
            if d == 0:
                return v
            o = np.zeros_like(v)
            if d > 0:
                o[d:] = v[:-d]
            else:
                o[:d] = v[-d:]
            return o

        def row(arr, j):
            return arr[j] if j in arr else z

        m_c = mm[r]
        w = np.zeros(HO, bool)
        for arr, (o1, o2) in ((a2, ((-1, 0), (1, 0))),
                              (a0, ((0, -1), (0, 1))),
                              (a3, ((-1, -1), (1, 1))),
                              (a1, ((-1, 1), (1, -1)))):
            cen = row(arr, r)
            # sh(v, d) yields o[c] = v[c - d]; neighbor at col c + dc
            # therefore needs d = -dc.
            n1 = sh(row(arr, r + o1[0]), -o1[1])
            n2 = sh(row(arr, r + o2[0]), -o2[1])
            w |= (cen >= np.maximum(np.maximum(n1, n2), f(MIN2)))
        w50[oi] = w.astype(np.uint8)
        big[oi] = (m_c >= f(MAX2)).astype(np.uint8)
    return w50, big


def _assemble_core(ou, im_pair):
    """ou: (NCH, NPART, 5, RPP, CW) u8 device maps for one core.
    im_pair: (2, 1024, 1024) f32 raw images.
    Returns (e_img, e_week, e_sure) each (2, HO, HO) f32."""
    idx = _pad_idx()
    o = ou.reshape(NCH, NIMG, PPI, 5, RPP, CW)
    o = o.transpose(1, 3, 2, 4, 0, 5).reshape(NIMG, 5, PPI * RPP,
                                              NCH * CW)[:, :, :, :HO]
    e_img = np.empty((NIMG, HO, HO), np.float32)
    e_week = np.empty((NIMG, HO, HO), np.float32)
    e_sure = np.empty((NIMG, HO, HO), np.float32)
    for k in range(NIMG):
        w50 = (o[k, 0] + o[k, 1] + o[k, 2] + o[k, 3])
        sure = w50 * o[k, 4]
        p = im_pair[k][idx][:, idx]
        sw, sb = _strip_rows(p)
        W = np.empty((HO, HO), np.float32)
        S = np.empty((HO, HO), np.float32)
        W[1:1024] = w50[0:1023]
        S[1:1024] = sure[0:1023]
        W[0] = sw[0]
        S[0] = sw[0] * sb[0]
        W[1024] = sw[1]
        S[1024] = sw[1] * sb[1]
        e_img[k] = W * np.float32(255.5)
        e_sure[k] = S * np.float32(255.0)
        e_week[k] = (W - S) * np.float32(255.0)
    return e_img, e_week, e_sure


def kernel(images):
    global LAST_RESULTS
    from concourse.bass_utils import run_bass_kernel_spmd

    images = np.asarray(images, dtype=np.float32)
    assert images.shape == (16, 1024, 1024, 1), images.shape
    im3 = images[:, :, :, 0]
    qcs = _build_qc(im3)

    nc = _get_nc()
    in_maps = [{"qc": qcs[c]} for c in range(8)]
    res = run_bass_kernel_spmd(nc, in_maps, list(range(8)))
    LAST_RESULTS = res

    e_img = np.empty((16, HO, HO, 1), np.float32)
    e_week = np.empty((16, HO, HO, 1), np.float32)
    e_sure = np.empty((16, HO, HO, 1), np.float32)
    for c in range(8):
        ei, ew, es = _assemble_core(res.results[c]["ou"],
                                    im3[c * NIMG:(c + 1) * NIMG])
        e_img[c * NIMG:(c + 1) * NIMG, :, :, 0] = ei
        e_week[c * NIMG:(c + 1) * NIMG, :, :, 0] = ew
        e_sure[c * NIMG:(c + 1) * NIMG, :, :, 0] = es
    return e_img, e_week, e_sure


# revision 38
# speedup vs baseline: 1.2126x; 1.2126x over previous
"""Canny edge detection (nn_CannyEdge_83330955477492) on 8 Trainium2 cores.

Pipeline reproduced from the reference:
  - The reference's "gaussian blur" (sigma=0.05, and a 2x2 kernel thanks to
    arange(-(3//2)+1, 3//2+1) == [0,1]) is exactly a top-left crop of the
    reflect-padded image: blur[i,j] = x[R(i-1), R(j-1)], R(-1)=1 -> 1025x1025.
  - Sobel gx/gy on the reflect-padded blur (correlation).
  - Direction binning by exact slope comparisons on the SQUARES
    (T^2*gx^2 vs gy^2) instead of atan2 (bit-identical except for pixels
    within ~1 ulp of a bin boundary).
  - Magnitude comparisons use gx^2+gy^2 (monotone equivalent of sqrt).
  - NMS via shifted maxes per bin; thresholds at 50^2/100^2.

Sharding: pure data parallel, 2 images per core.

Layout: 128 partitions (64 per image), 16 output rows per partition,
9 column chunks of 116.  The device emits five u8 indicator maps (4 per-bin
weak-edge bits + the >=100 bit) packed in one contiguous store per chunk;
the host sums/crops them into the three 0/255 float maps.  Image rows 0 and
1024 (which see the NMS zero-padding) are recomputed exactly on the host,
which removes all border masking from the device inner loop.

Work is split across three engines, restricted to what the real walrus
backend accepts per engine (Pool = gpsimd: tensor_tensor add/sub/mult and
tensor_scalar only, SBUF only; ACT: 1-input affine+func; DVE: everything):
  Pool: all adds/subs/mults (tt, rsm/gx assembly, dd, t2, gy, sg, mm, the
        masked-magnitude builds) + two bins' weak-bit chains via the exact
        sign-of-subtract trick  w = [cen - max(qt,50^2) >= 0].
  ACT:  the 2*center scaled copies and the gradient squares.
  DVE:  all compares (bin masks, NMS maxes, weak/sure bits).
All arithmetic is plain fp32, bit-identical to the reference up to ulp-level
comparison ties (the fp32r tensor-engine path was rejected: real-HW fp32r
rounds its operands, flipping ~5k near-tie NMS comparisons).
"""
import numpy as np

# ---------------------------------------------------------------- geometry
NIMG = 2             # images per core
H = 1024             # input image size
HO = 1025            # output size (blur is 1025x1025)
RPP = 16             # output rows per partition
PPI = 64             # partitions per image
NPART = NIMG * PPI   # 128
CW = 116             # output cols per chunk
NCH = 9              # chunks (9*116 = 1044 >= 1025)
WA = CW + 4          # loaded cols per chunk (even: fp32r matmul spans)
WB = CW + 2          # stencil cols per chunk (even)
TROWS = 21           # tin rows (20 + 1 pad so shifted flat spans stay in)
QROWS_IMG = 1029     # P's 1027 rows + 2 zero pad
QCOLS = NCH * CW + WA - CW  # 1 zero col + P's 1027 cols + pad
LASTC = HO - CW * (NCH - 1) + 1  # mm col of blur col 1025 in last chunk

_T1 = np.float32(np.tan(np.deg2rad(22.5)))
_T2 = np.float32(np.tan(np.deg2rad(67.5)))
T1S = float(np.float32(_T1 * _T1))
T2S = float(np.float32(_T2 * _T2))
MIN2 = float(np.float32(50.0 * 50.0))
MAX2 = float(np.float32(100.0 * 100.0))

_NC = None
LAST_RESULTS = None  # stashed BassKernelResults for test.py


# ------------------------------------------------- walrus 1-wait workaround
def _split_multiwaits(nc):
    """This walrus build rejects >1 sync wait per instruction: move extra
    waits onto fresh same-engine NOPs inserted right before the carrier."""
    import concourse.mybir as mybir

    n_split = 0
    for fn in nc.m.functions:
        for bb in fn.blocks:
            insts = list(bb.instructions)
            if not any(
                i.sync_info is not None
                and i.sync_info.on_wait
                and len(i.sync_info.on_wait) > 1
                for i in insts
            ):
                continue
            out = []
            for inst in insts:
                si = inst.sync_info
                if si is not None and si.on_wait and len(si.on_wait) > 1:
                    waits = list(si.on_wait)
                    eng = nc.engines[inst.engine]
                    for w in waits[:-1]:
                        nop = eng.nop(hint="waitsplit")
                        # eng.nop() appended to nc.cur_bb — remove it there
                        # (it must live ONLY at its split position, else the
                        # duplicate runs after sem cleanup and deadlocks).
                        host = nc.cur_bb.bb
                        lst = list(host.instructions)
                        assert lst and lst[-1].name == nop.ins.name
                        _set_insts(host, lst[:-1])
                        nop.ins.sync_info = mybir.SyncInfo(
                            on_wait=[w], on_update=[]
                        )
                        out.append(nop.ins)
                        n_split += 1
                    si.on_wait = waits[-1:]
                out.append(inst)
            _set_insts(bb, out)
    return n_split


def _set_insts(bb, lst):
    try:
        bb.instructions = lst
    except Exception:
        bb.instructions.clear()
        bb.instructions.extend(lst)


# ------------------------------------------------------------ device build
#
# v3: the PE (tensor engine) computes every pure add/sub of (shifted) views
# as float32r identity matmuls accumulated in PSUM (bit-exact: products are
# 1.0*x or 2.0*x with exact fp32 PSUM accumulation).  Each matmul output is
# one PSUM bank (3 rows x WB = 393 f32 <= 512); tensors are processed in two
# row-halves so two 4-bank PSUM slots ping-pong.  ACT drains PSUM into dense
# SBUF tiles (fused with Square for gx/gy); DVE/Pool keep the tensor
# multiplies and all NMS compares.
def _build_nc():
    import concourse.bass as bass
    import concourse.tile as tile
    import concourse.mybir as mybir
    from concourse.ap import AP

    f32 = mybir.dt.float32
    f32r = mybir.dt.float32r
    u8 = mybir.dt.uint8
    Alu = mybir.AluOpType
    Act = mybir.ActivationFunctionType

    nc = bass.Bass("TRN2", target_bir_lowering=False, debug=False, num_devices=8)
    qc = nc.declare_dram_parameter("qc", [NCH, NPART, TROWS, WA], f32,
                                   isOutput=False)
    ou = nc.declare_dram_parameter("ou", [NCH, NPART, 5, RPP, CW], u8,
                                   isOutput=True)

    with tile.TileContext(nc) as tc:
        with (
            tc.tile_pool(name="io2", bufs=2) as io2,   # load/store overlap
            tc.tile_pool(name="mid", bufs=1) as mid,   # per-chunk work tiles
        ):
            def stage1(ci):
                """Load + row stencils for chunk ci.  rsm is accumulated in
                place (ACT writes 2*C, Pool adds L then R) so no scratch
                tiles are needed; dd = R - L on Pool."""
                tin = io2.tile([NPART, TROWS, WA], f32, tag="tin")
                src = AP(qc, ci * NPART * TROWS * WA,
                         [[TROWS * WA, NPART], [WA, TROWS], [1, WA]])
                nc.sync.dma_start(out=tin[:], in_=src)

                rsm = mid.tile([NPART, 20, WB], f32, tag="rsm", bufs=2)
                nc.scalar.activation(out=rsm[:], in_=tin[:, 0:20, 1:WB + 1],
                                     func=Act.Copy, scale=2.0)
                nc.gpsimd.tensor_tensor(out=rsm[:], in0=rsm[:],
                                        in1=tin[:, 0:20, 0:WB], op=Alu.add)
                nc.gpsimd.tensor_tensor(out=rsm[:], in0=rsm[:],
                                        in1=tin[:, 0:20, 2:WA], op=Alu.add)
                dd = mid.tile([NPART, 20, WB], f32, tag="dd", bufs=2)
                nc.gpsimd.tensor_tensor(
                    out=dd[:], in0=tin[:, 0:20, 2:WA], in1=tin[:, 0:20, 0:WB],
                    op=Alu.subtract)
                return dict(rsm=rsm, dd=dd)

            def stage2(ci, st):
                """Gradients, sign, squares, bin masks, magnitude."""
                rsm, dd = st["rsm"], st["dd"]
                gx = mid.tile([NPART, 18, WB], f32, tag="gx")
                nc.scalar.activation(out=gx[:], in_=dd[:, 1:19, :],
                                     func=Act.Copy, scale=2.0)
                nc.gpsimd.tensor_tensor(out=gx[:], in0=gx[:],
                                        in1=dd[:, 0:18, :], op=Alu.add)
                nc.gpsimd.tensor_tensor(out=gx[:], in0=gx[:],
                                        in1=dd[:, 2:20, :], op=Alu.add)
                gy = mid.tile([NPART, 18, WB], f32, tag="gy")
                nc.gpsimd.tensor_tensor(
                    out=gy[:], in0=rsm[:, 2:20, :], in1=rsm[:, 0:18, :],
                    op=Alu.subtract)
                sg = mid.tile([NPART, 18, WB], f32, tag="sg", bufs=2)
                nc.gpsimd.tensor_tensor(out=sg[:], in0=gx[:], in1=gy[:],
                                        op=Alu.mult)
                # squares in place (ACT)
                nc.scalar.activation(out=gx[:], in_=gx[:], func=Act.Square)
                nc.scalar.activation(out=gy[:], in_=gy[:], func=Act.Square)

                # bin masks from the squares (DVE)
                c0 = mid.tile([NPART, 18, WB], f32, tag="c0", bufs=2)
                nc.vector.scalar_tensor_tensor(
                    out=c0[:], in0=gx[:], scalar=T1S, in1=gy[:],
                    op0=Alu.mult, op1=Alu.is_ge)
                d2 = mid.tile([NPART, 18, WB], f32, tag="d2", bufs=2)
                nc.vector.scalar_tensor_tensor(
                    out=d2[:], in0=gx[:], scalar=T2S, in1=gy[:],
                    op0=Alu.mult, op1=Alu.is_gt)

                # magnitude^2 + NMS zero-pad cols at image edges
                mm = mid.tile([NPART, 18, WB], f32, tag="mm", bufs=2)
                nc.gpsimd.tensor_tensor(out=mm[:], in0=gx[:], in1=gy[:],
                                        op=Alu.add)
                if ci == 0:
                    nc.gpsimd.memset(mm[:, :, 0:1], 0.0)       # blur col -1
                if ci == NCH - 1:
                    nc.gpsimd.memset(mm[:, :, LASTC:LASTC + 1], 0.0)
                return dict(sg=sg, c0=c0, d2=d2, mm=mm)

            def stage3(ci, st):
                """NMS per-bin processing + store for chunk ci."""
                sg, c0, d2, mm = st["sg"], st["c0"], st["d2"], st["mm"]
                ws = io2.tile([NPART, 5, RPP, CW], u8, tag="ws")

                def wslot(s):
                    return ws[:, s:s + 1].rearrange("p a r c -> p (a r) c")

                nc.vector.tensor_scalar(
                    out=wslot(4), in0=mm[:, 1:17, 1:1 + CW], scalar1=MAX2,
                    scalar2=None, op0=Alu.is_ge)

                def nms_bin(ang, r1, c1, r2, c2, slot):
                    qt = mid.tile([NPART, RPP, CW], f32, tag="qt")
                    nc.vector.tensor_tensor(
                        out=qt[:],
                        in0=ang[:, r1:r1 + RPP, c1:c1 + CW],
                        in1=ang[:, r2:r2 + RPP, c2:c2 + CW],
                        op=Alu.max)
                    nc.vector.scalar_tensor_tensor(
                        out=wslot(slot), in0=qt[:], scalar=MIN2,
                        in1=ang[:, 1:17, 1:1 + CW], op0=Alu.max,
                        op1=Alu.is_le)

                # bins are disjoint; each w bit is 1 only where the center
                # is in the bin and >= max(neighbors, 50^2).
                md2 = mid.tile([NPART, 18, WB], f32, tag="md2")
                nc.gpsimd.tensor_tensor(out=md2[:], in0=mm[:], in1=d2[:],
                                        op=Alu.mult)      # Md2 = M*d2
                angA = mid.tile([NPART, 18, WB], f32, tag="angA")
                nc.gpsimd.tensor_tensor(out=angA[:], in0=mm[:], in1=md2[:],
                                        op=Alu.subtract)  # ang2 = M - Md2
                nms_bin(angA, 0, 1, 2, 1, 2)              # bin2: up/down
                angB = mid.tile([NPART, 18, WB], f32, tag="angB")
                nc.gpsimd.tensor_tensor(out=angB[:], in0=md2[:], in1=c0[:],
                                        op=Alu.mult)      # ang0 = Md2*c0
                nms_bin(angB, 1, 0, 1, 2, 0)              # bin0: left/right
                mdg = mid.tile([NPART, 18, WB], f32, tag="angA")
                nc.gpsimd.tensor_tensor(out=mdg[:], in0=md2[:], in1=angB[:],
                                        op=Alu.subtract)  # mdiag
                angC = mid.tile([NPART, 18, WB], f32, tag="angC")
                nc.vector.scalar_tensor_tensor(
                    out=angC[:], in0=sg[:], scalar=0.0, in1=mdg[:],
                    op0=Alu.is_gt, op1=Alu.mult)          # ang3 = (s>0)*mdiag
                nms_bin(angC, 0, 0, 2, 2, 3)              # bin3: main diag
                ang1 = mid.tile([NPART, 18, WB], f32, tag="angB")
                nc.gpsimd.tensor_tensor(out=ang1[:], in0=mdg[:],
                                        in1=angC[:], op=Alu.subtract)
                nms_bin(ang1, 0, 2, 2, 0, 1)              # bin1: anti diag

                dst = AP(ou, ci * NPART * 5 * RPP * CW,
                         [[5 * RPP * CW, NPART], [RPP * CW, 5],
                          [CW, RPP], [1, CW]])
                nc.sync.dma_start(out=dst, in_=ws[:])

            # 3-deep software pipeline
            s1 = {}
            s2 = {}
            for ci in range(NCH + 2):
                if 1 <= ci <= NCH:
                    s2[ci - 1] = stage2(ci - 1, s1.pop(ci - 1))
                if ci < NCH:
                    s1[ci] = stage1(ci)
                if ci >= 2:
                    stage3(ci - 2, s2.pop(ci - 2))

    _split_multiwaits(nc)
    return nc


def _get_nc():
    global _NC
    if _NC is None:
        _NC = _build_nc()
    return _NC


# ------------------------------------------------------------- host helpers
def _eye3():
    """Stationary matrices for the PE identity matmuls: I, 2I, -I."""
    e = np.eye(NPART, dtype=np.float32)
    return np.stack([e, 2.0 * e, -e], axis=1).astype(np.float32)


def _pad_idx():
    idx = np.empty(1027, np.int64)
    idx[0] = 0
    idx[1] = 1
    idx[2:1026] = np.arange(1024)
    idx[1026] = 1022
    return idx


def _build_qc(images):
    """images: (16, 1024, 1024) f32 -> per-core chunked input
    (8, NCH, NPART, 20, WA) with all halos baked in so every load is one
    contiguous descriptor per partition."""
    idx = _pad_idx()
    qcs = np.zeros((8, NCH, NPART, TROWS, WA), np.float32)
    win = np.zeros((NPART, TROWS, QCOLS), np.float32)
    for core in range(8):
        for k in range(NIMG):
            im = images[core * NIMG + k]
            p = im[idx][:, idx]                      # (1027, 1027) = bp rows
            q = np.zeros((QROWS_IMG, QCOLS), np.float32)
            q[0:1027, 1:1028] = p
            # partition p' takes rows 16p'..16p'+20
            sw = np.lib.stride_tricks.sliding_window_view(q, TROWS, axis=0)
            win[k * PPI:(k + 1) * PPI] = sw[0:16 * PPI:16].transpose(0, 2, 1)
        for ci in range(NCH):
            qcs[core, ci] = win[:, :, CW * ci:CW * ci + WA]
    return qcs


def _strip_rows(p):
    """Exact f32 Canny decision bits for output rows 0 and 1024.

    p: (1027, 1027) f32 padded blur (rows/cols -1..1025).
    Returns (w50, big) as uint8 arrays of shape (2, 1025)."""
    f = np.float32
    w50 = np.zeros((2, HO), np.uint8)
    big = np.zeros((2, HO), np.uint8)
    for oi, r in enumerate((0, H)):
        rows = [j for j in (r - 1, r, r + 1) if 0 <= j <= H]
        mm = {}
        a2 = {}
        a0 = {}
        a3 = {}
        a1 = {}
        for j in rows:
            s = p[j:j + 3]                           # 3 x 1027
            ddr = s[:, 2:] - s[:, :-2]               # 3 x 1025
            ssr = s[:, :-2] + f(2.0) * s[:, 1:-1] + s[:, 2:]
            gx = ddr[0] + f(2.0) * ddr[1] + ddr[2]
            gy = ssr[2] - ssr[0]
            m = gx * gx + gy * gy
            ax = np.abs(gx)
            ay = np.abs(gy)
            c0 = (_T1 * ax >= ay)
            d2 = (_T2 * ax > ay)
            sgp = (gx * gy) > 0
            mm[j] = m
            a2[j] = np.where(d2, f(0), m)
            md = np.where(d2, m, f(0))
            a0[j] = np.where(c0, md, f(0))
            mdg = md - a0[j]
            a3[j] = np.where(sgp, mdg, f(0))
            a1[j] = mdg - a3[j]
        z = np.zeros(HO, np.float32)

        def sh(v, d):  # shift cols by d with zero pad
            if d == 0:
                return v
            o = np.zeros_like(v)
            if d > 0:
                o[d:] = v[:-d]
            else:
                o[:d] = v[-d:]
            return o

        def row(arr, j):
            return arr[j] if j in arr else z

        m_c = mm[r]
        w = np.zeros(HO, bool)
        for arr, (o1, o2) in ((a2, ((-1, 0), (1, 0))),
                              (a0, ((0, -1), (0, 1))),
                              (a3, ((-1, -1), (1, 1))),
                              (a1, ((-1, 1), (1, -1)))):
            cen = row(arr, r)
            # sh(v, d) yields o[c] = v[c - d]; neighbor at col c + dc
            # therefore needs d = -dc.
            n1 = sh(row(arr, r + o1[0]), -o1[1])
            n2 = sh(row(arr, r + o2[0]), -o2[1])
            w |= (cen >= np.maximum(np.maximum(n1, n2), f(MIN2)))
        w50[oi] = w.astype(np.uint8)
        big[oi] = (m_c >= f(MAX2)).astype(np.uint8)
    return w50, big


def _assemble_core(ou, im_pair):
    """ou: (NCH, NPART, 5, RPP, CW) u8 device maps for one core.
    im_pair: (2, 1024, 1024) f32 raw images.
    Returns (e_img, e_week, e_sure) each (2, HO, HO) f32."""
    idx = _pad_idx()
    o = ou.reshape(NCH, NIMG, PPI, 5, RPP, CW)
    o = o.transpose(1, 3, 2, 4, 0, 5).reshape(NIMG, 5, PPI * RPP,
                                              NCH * CW)[:, :, :, :HO]
    e_img = np.empty((NIMG, HO, HO), np.float32)
    e_week = np.empty((NIMG, HO, HO), np.float32)
    e_sure = np.empty((NIMG, HO, HO), np.float32)
    for k in range(NIMG):
        w50 = (o[k, 0] + o[k, 1] + o[k, 2] + o[k, 3])
        sure = w50 * o[k, 4]
        p = im_pair[k][idx][:, idx]
        sw, sb = _strip_rows(p)
        W = np.empty((HO, HO), np.float32)
        S = np.empty((HO, HO), np.float32)
        W[1:1024] = w50[0:1023]
        S[1:1024] = sure[0:1023]
        W[0] = sw[0]
        S[0] = sw[0] * sb[0]
        W[1024] = sw[1]
        S[1024] = sw[1] * sb[1]
        e_img[k] = W * np.float32(255.5)
        e_sure[k] = S * np.float32(255.0)
        e_week[k] = (W - S) * np.float32(255.0)
    return e_img, e_week, e_sure


def kernel(images):
    global LAST_RESULTS
    from concourse.bass_utils import run_bass_kernel_spmd

    images = np.asarray(images, dtype=np.float32)
    assert images.shape == (16, 1024, 1024, 1), images.shape
    im3 = images[:, :, :, 0]
    qcs = _build_qc(im3)

    nc = _get_nc()
    in_maps = [{"qc": qcs[c]} for c in range(8)]
    res = run_bass_kernel_spmd(nc, in_maps, list(range(8)))
    LAST_RESULTS = res

    e_img = np.empty((16, HO, HO, 1), np.float32)
    e_week = np.empty((16, HO, HO, 1), np.float32)
    e_sure = np.empty((16, HO, HO, 1), np.float32)
    for c in range(8):
        ei, ew, es = _assemble_core(res.results[c]["ou"],
                                    im3[c * NIMG:(c + 1) * NIMG])
        e_img[c * NIMG:(c + 1) * NIMG, :, :, 0] = ei
        e_week[c * NIMG:(c + 1) * NIMG, :, :, 0] = ew
        e_sure[c * NIMG:(c + 1) * NIMG, :, :, 0] = es
    return e_img, e_week, e_sure


# revision 43
# speedup vs baseline: 1.2260x; 1.0111x over previous
"""Canny edge detection (nn_CannyEdge_83330955477492) on 8 Trainium2 cores.

Pipeline reproduced from the reference:
  - The reference's "gaussian blur" (sigma=0.05, and a 2x2 kernel thanks to
    arange(-(3//2)+1, 3//2+1) == [0,1]) is exactly a top-left crop of the
    reflect-padded image: blur[i,j] = x[R(i-1), R(j-1)], R(-1)=1 -> 1025x1025.
  - Sobel gx/gy on the reflect-padded blur (correlation).
  - Direction binning by exact slope comparisons on the SQUARES
    (T^2*gx^2 vs gy^2) instead of atan2 (bit-identical except for pixels
    within ~1 ulp of a bin boundary).
  - Magnitude comparisons use gx^2+gy^2 (monotone equivalent of sqrt).
  - NMS via shifted maxes per bin; thresholds at 50^2/100^2.

Sharding: pure data parallel, 2 images per core.

Layout: 128 partitions (64 per image), 16 output rows per partition,
9 column chunks of 116.  The device emits five u8 indicator maps (4 per-bin
weak-edge bits + the >=100 bit) packed in one contiguous store per chunk;
the host sums/crops them into the three 0/255 float maps.  Image rows 0 and
1024 (which see the NMS zero-padding) are recomputed exactly on the host,
which removes all border masking from the device inner loop.

Work is split across three engines, restricted to what the real walrus
backend accepts per engine (Pool = gpsimd: tensor_tensor add/sub/mult and
tensor_scalar only, SBUF only; ACT: 1-input affine+func; DVE: everything):
  Pool: all adds/subs/mults (tt, rsm/gx assembly, dd, t2, gy, sg, mm, the
        masked-magnitude builds) + two bins' weak-bit chains via the exact
        sign-of-subtract trick  w = [cen - max(qt,50^2) >= 0].
  ACT:  the 2*center scaled copies and the gradient squares.
  DVE:  all compares (bin masks, NMS maxes, weak/sure bits).
All arithmetic is plain fp32, bit-identical to the reference up to ulp-level
comparison ties (the fp32r tensor-engine path was rejected: real-HW fp32r
rounds its operands, flipping ~5k near-tie NMS comparisons).
"""
import numpy as np

# ---------------------------------------------------------------- geometry
NIMG = 2             # images per core
H = 1024             # input image size
HO = 1025            # output size (blur is 1025x1025)
RPP = 16             # output rows per partition
PPI = 64             # partitions per image
NPART = NIMG * PPI   # 128
CW = 116             # output cols per chunk
NCH = 9              # chunks (9*116 = 1044 >= 1025)
WA = CW + 4          # loaded cols per chunk (even: fp32r matmul spans)
WB = CW + 2          # stencil cols per chunk (even)
TROWS = 21           # tin rows (20 + 1 pad so shifted flat spans stay in)
QROWS_IMG = 1029     # P's 1027 rows + 2 zero pad
QCOLS = NCH * CW + WA - CW  # 1 zero col + P's 1027 cols + pad
LASTC = HO - CW * (NCH - 1) + 1  # mm col of blur col 1025 in last chunk

_T1 = np.float32(np.tan(np.deg2rad(22.5)))
_T2 = np.float32(np.tan(np.deg2rad(67.5)))
T1S = float(np.float32(_T1 * _T1))
T2S = float(np.float32(_T2 * _T2))
MIN2 = float(np.float32(50.0 * 50.0))
MAX2 = float(np.float32(100.0 * 100.0))

_NC = None
LAST_RESULTS = None  # stashed BassKernelResults for test.py


# ------------------------------------------------- walrus 1-wait workaround
def _split_multiwaits(nc):
    """This walrus build rejects >1 sync wait per instruction: move extra
    waits onto fresh same-engine NOPs inserted right before the carrier."""
    import concourse.mybir as mybir

    n_split = 0
    for fn in nc.m.functions:
        for bb in fn.blocks:
            insts = list(bb.instructions)
            if not any(
                i.sync_info is not None
                and i.sync_info.on_wait
                and len(i.sync_info.on_wait) > 1
                for i in insts
            ):
                continue
            out = []
            for inst in insts:
                si = inst.sync_info
                if si is not None and si.on_wait and len(si.on_wait) > 1:
                    waits = list(si.on_wait)
                    eng = nc.engines[inst.engine]
                    for w in waits[:-1]:
                        nop = eng.nop(hint="waitsplit")
                        # eng.nop() appended to nc.cur_bb — remove it there
                        # (it must live ONLY at its split position, else the
                        # duplicate runs after sem cleanup and deadlocks).
                        host = nc.cur_bb.bb
                        lst = list(host.instructions)
                        assert lst and lst[-1].name == nop.ins.name
                        _set_insts(host, lst[:-1])
                        nop.ins.sync_info = mybir.SyncInfo(
                            on_wait=[w], on_update=[]
                        )
                        out.append(nop.ins)
                        n_split += 1
                    si.on_wait = waits[-1:]
                out.append(inst)
            _set_insts(bb, out)
    return n_split


def _set_insts(bb, lst):
    try:
        bb.instructions = lst
    except Exception:
        bb.instructions.clear()
        bb.instructions.extend(lst)


# ------------------------------------------------------------ device build
#
# v3: the PE (tensor engine) computes every pure add/sub of (shifted) views
# as float32r identity matmuls accumulated in PSUM (bit-exact: products are
# 1.0*x or 2.0*x with exact fp32 PSUM accumulation).  Each matmul output is
# one PSUM bank (3 rows x WB = 393 f32 <= 512); tensors are processed in two
# row-halves so two 4-bank PSUM slots ping-pong.  ACT drains PSUM into dense
# SBUF tiles (fused with Square for gx/gy); DVE/Pool keep the tensor
# multiplies and all NMS compares.
def _build_nc():
    import concourse.bass as bass
    import concourse.tile as tile
    import concourse.mybir as mybir
    from concourse.ap import AP

    f32 = mybir.dt.float32
    f32r = mybir.dt.float32r
    u8 = mybir.dt.uint8
    Alu = mybir.AluOpType
    Act = mybir.ActivationFunctionType

    nc = bass.Bass("TRN2", target_bir_lowering=False, debug=False, num_devices=8)
    qc = nc.declare_dram_parameter("qc", [NCH, NPART, TROWS, WA], f32,
                                   isOutput=False)
    ou = nc.declare_dram_parameter("ou", [NCH, NPART, 5, RPP, CW], u8,
                                   isOutput=True)

    with tile.TileContext(nc) as tc:
        with (
            tc.tile_pool(name="io2", bufs=2) as io2,   # load/store overlap
            tc.tile_pool(name="mid", bufs=1) as mid,   # per-chunk work tiles
        ):
            def stage1(ci):
                """Load + row stencils for chunk ci.  rsm is accumulated in
                place (ACT writes 2*C, Pool adds L then R) so no scratch
                tiles are needed; dd = R - L on Pool."""
                tin = io2.tile([NPART, TROWS, WA], f32, tag="tin")
                src = AP(qc, ci * NPART * TROWS * WA,
                         [[TROWS * WA, NPART], [WA, TROWS], [1, WA]])
                nc.sync.dma_start(out=tin[:], in_=src)

                rsm = mid.tile([NPART, 20, WB], f32, tag="rsm", bufs=2)
                nc.scalar.activation(out=rsm[:], in_=tin[:, 0:20, 1:WB + 1],
                                     func=Act.Copy, scale=2.0)
                nc.gpsimd.tensor_tensor(out=rsm[:], in0=rsm[:],
                                        in1=tin[:, 0:20, 0:WB], op=Alu.add)
                nc.gpsimd.tensor_tensor(out=rsm[:], in0=rsm[:],
                                        in1=tin[:, 0:20, 2:WA], op=Alu.add)
                dd = mid.tile([NPART, 20, WB], f32, tag="dd", bufs=2)
                nc.gpsimd.tensor_tensor(
                    out=dd[:], in0=tin[:, 0:20, 2:WA], in1=tin[:, 0:20, 0:WB],
                    op=Alu.subtract)
                return dict(rsm=rsm, dd=dd)

            def stage2(ci, st):
                """Gradients, sign, squares, bin masks, magnitude."""
                rsm, dd = st["rsm"], st["dd"]
                gx = mid.tile([NPART, 18, WB], f32, tag="gx")
                nc.scalar.activation(out=gx[:], in_=dd[:, 1:19, :],
                                     func=Act.Copy, scale=2.0)
                nc.gpsimd.tensor_tensor(out=gx[:], in0=gx[:],
                                        in1=dd[:, 0:18, :], op=Alu.add)
                nc.gpsimd.tensor_tensor(out=gx[:], in0=gx[:],
                                        in1=dd[:, 2:20, :], op=Alu.add)
                gy = mid.tile([NPART, 18, WB], f32, tag="gy")
                nc.gpsimd.tensor_tensor(
                    out=gy[:], in0=rsm[:, 2:20, :], in1=rsm[:, 0:18, :],
                    op=Alu.subtract)
                sg = mid.tile([NPART, 18, WB], f32, tag="sg", bufs=2)
                nc.gpsimd.tensor_tensor(out=sg[:], in0=gx[:], in1=gy[:],
                                        op=Alu.mult)
                # squares in place (ACT)
                nc.scalar.activation(out=gx[:], in_=gx[:], func=Act.Square)
                nc.scalar.activation(out=gy[:], in_=gy[:], func=Act.Square)

                # bin masks from the squares (DVE)
                c0 = mid.tile([NPART, 18, WB], f32, tag="c0", bufs=2)
                nc.vector.scalar_tensor_tensor(
                    out=c0[:], in0=gx[:], scalar=T1S, in1=gy[:],
                    op0=Alu.mult, op1=Alu.is_ge)
                d2 = mid.tile([NPART, 18, WB], f32, tag="d2", bufs=2)
                nc.vector.scalar_tensor_tensor(
                    out=d2[:], in0=gx[:], scalar=T2S, in1=gy[:],
                    op0=Alu.mult, op1=Alu.is_gt)

                # magnitude^2 + NMS zero-pad cols at image edges
                mm = mid.tile([NPART, 18, WB], f32, tag="mm", bufs=2)
                nc.gpsimd.tensor_tensor(out=mm[:], in0=gx[:], in1=gy[:],
                                        op=Alu.add)
                if ci == 0:
                    nc.gpsimd.memset(mm[:, :, 0:1], 0.0)       # blur col -1
                if ci == NCH - 1:
                    nc.gpsimd.memset(mm[:, :, LASTC:LASTC + 1], 0.0)
                return dict(sg=sg, c0=c0, d2=d2, mm=mm)

            def stage3(ci, st):
                """NMS per-bin processing + store for chunk ci."""
                sg, c0, d2, mm = st["sg"], st["c0"], st["d2"], st["mm"]
                ws = io2.tile([NPART, 5, RPP, CW], u8, tag="ws")

                def wslot(s):
                    return ws[:, s:s + 1].rearrange("p a r c -> p (a r) c")

                nc.vector.tensor_scalar(
                    out=wslot(4), in0=mm[:, 1:17, 1:1 + CW], scalar1=MAX2,
                    scalar2=None, op0=Alu.is_ge)

                def nms_bin(ang, r1, c1, r2, c2, slot):
                    qt = mid.tile([NPART, RPP, CW], f32, tag="qt", bufs=2)
                    nc.vector.tensor_tensor(
                        out=qt[:],
                        in0=ang[:, r1:r1 + RPP, c1:c1 + CW],
                        in1=ang[:, r2:r2 + RPP, c2:c2 + CW],
                        op=Alu.max)
                    nc.vector.scalar_tensor_tensor(
                        out=wslot(slot), in0=qt[:], scalar=MIN2,
                        in1=ang[:, 1:17, 1:1 + CW], op0=Alu.max,
                        op1=Alu.is_le)

                # bins are disjoint; each w bit is 1 only where the center
                # is in the bin and >= max(neighbors, 50^2).
                md2 = mid.tile([NPART, 18, WB], f32, tag="md2")
                nc.gpsimd.tensor_tensor(out=md2[:], in0=mm[:], in1=d2[:],
                                        op=Alu.mult)      # Md2 = M*d2
                angA = mid.tile([NPART, 18, WB], f32, tag="angA")
                nc.gpsimd.tensor_tensor(out=angA[:], in0=mm[:], in1=md2[:],
                                        op=Alu.subtract)  # ang2 = M - Md2
                nms_bin(angA, 0, 1, 2, 1, 2)              # bin2: up/down
                angB = mid.tile([NPART, 18, WB], f32, tag="angB")
                nc.gpsimd.tensor_tensor(out=angB[:], in0=md2[:], in1=c0[:],
                                        op=Alu.mult)      # ang0 = Md2*c0
                nms_bin(angB, 1, 0, 1, 2, 0)              # bin0: left/right
                mdg = mid.tile([NPART, 18, WB], f32, tag="angA")
                nc.gpsimd.tensor_tensor(out=mdg[:], in0=md2[:], in1=angB[:],
                                        op=Alu.subtract)  # mdiag
                angC = mid.tile([NPART, 18, WB], f32, tag="angC")
                nc.vector.scalar_tensor_tensor(
                    out=angC[:], in0=sg[:], scalar=0.0, in1=mdg[:],
                    op0=Alu.is_gt, op1=Alu.mult)          # ang3 = (s>0)*mdiag
                nms_bin(angC, 0, 0, 2, 2, 3)              # bin3: main diag
                ang1 = mid.tile([NPART, 18, WB], f32, tag="angB")
                nc.gpsimd.tensor_tensor(out=ang1[:], in0=mdg[:],
                                        in1=angC[:], op=Alu.subtract)
                nms_bin(ang1, 0, 2, 2, 0, 1)              # bin1: anti diag

                dst = AP(ou, ci * NPART * 5 * RPP * CW,
                         [[5 * RPP * CW, NPART], [RPP * CW, 5],
                          [CW, RPP], [1, CW]])
                nc.sync.dma_start(out=dst, in_=ws[:])

            # 3-deep software pipeline
            s1 = {}
            s2 = {}
            for ci in range(NCH + 2):
                if 1 <= ci <= NCH:
                    s2[ci - 1] = stage2(ci - 1, s1.pop(ci - 1))
                if ci >= 2:
                    stage3(ci - 2, s2.pop(ci - 2))
                if ci < NCH:
                    s1[ci] = stage1(ci)

    _split_multiwaits(nc)
    return nc


def _get_nc():
    global _NC
    if _NC is None:
        _NC = _build_nc()
    return _NC


# ------------------------------------------------------------- host helpers
def _eye3():
    """Stationary matrices for the PE identity matmuls: I, 2I, -I."""
    e = np.eye(NPART, dtype=np.float32)
    return np.stack([e, 2.0 * e, -e], axis=1).astype(np.float32)


def _pad_idx():
    idx = np.empty(1027, np.int64)
    idx[0] = 0
    idx[1] = 1
    idx[2:1026] = np.arange(1024)
    idx[1026] = 1022
    return idx


def _build_qc(images):
    """images: (16, 1024, 1024) f32 -> per-core chunked input
    (8, NCH, NPART, 20, WA) with all halos baked in so every load is one
    contiguous descriptor per partition."""
    idx = _pad_idx()
    qcs = np.zeros((8, NCH, NPART, TROWS, WA), np.float32)
    win = np.zeros((NPART, TROWS, QCOLS), np.float32)
    for core in range(8):
        for k in range(NIMG):
            im = images[core * NIMG + k]
            p = im[idx][:, idx]                      # (1027, 1027) = bp rows
            q = np.zeros((QROWS_IMG, QCOLS), np.float32)
            q[0:1027, 1:1028] = p
            # partition p' takes rows 16p'..16p'+20
            sw = np.lib.stride_tricks.sliding_window_view(q, TROWS, axis=0)
            win[k * PPI:(k + 1) * PPI] = sw[0:16 * PPI:16].transpose(0, 2, 1)
        for ci in range(NCH):
            qcs[core, ci] = win[:, :, CW * ci:CW * ci + WA]
    return qcs


def _strip_rows(p):
    """Exact f32 Canny decision bits for output rows 0 and 1024.

    p: (1027, 1027) f32 padded blur (rows/cols -1..1025).
    Returns (w50, big) as uint8 arrays of shape (2, 1025)."""
    f = np.float32
    w50 = np.zeros((2, HO), np.uint8)
    big = np.zeros((2, HO), np.uint8)
    for oi, r in enumerate((0, H)):
        rows = [j for j in (r - 1, r, r + 1) if 0 <= j <= H]
        mm = {}
        a2 = {}
        a0 = {}
        a3 = {}
        a1 = {}
        for j in rows:
            s = p[j:j + 3]                           # 3 x 1027
            ddr = s[:, 2:] - s[:, :-2]               # 3 x 1025
            ssr = s[:, :-2] + f(2.0) * s[:, 1:-1] + s[:, 2:]
            gx = ddr[0] + f(2.0) * ddr[1] + ddr[2]
            gy = ssr[2] - ssr[0]
            m = gx * gx + gy * gy
            ax = np.abs(gx)
            ay = np.abs(gy)
            c0 = (_T1 * ax >= ay)
            d2 = (_T2 * ax > ay)
            sgp = (gx * gy) > 0
            mm[j] = m
            a2[j] = np.where(d2, f(0), m)
            md = np.where(d2, m, f(0))
            a0[j] = np.where(c0, md, f(0))
            mdg = md - a0[j]
            a3[j] = np.where(sgp, mdg, f(0))
            a1[j] = mdg - a3[j]
        z = np.zeros(HO, np.float32)

        def sh(v, d):  # shift cols by d with zero pad
            if d == 0:
                return v
            o = np.zeros_like(v)
            if d > 0:
                o[d:] = v[:-d]
            else:
                o[:d] = v[-d:]
            return o

        def row(arr, j):
            return arr[j] if j in arr else z

        m_c = mm[r]
        w = np.zeros(HO, bool)
        for arr, (o1, o2) in ((a2, ((-1, 0), (1, 0))),
                              (a0, ((0, -1), (0, 1))),
                              (a3, ((-1, -1), (1, 1))),
                              (a1, ((-1, 1), (1, -1)))):
            cen = row(arr, r)
            # sh(v, d) yields o[c] = v[c - d]; neighbor at col c + dc
            # therefore needs d = -dc.
            n1 = sh(row(arr, r + o1[0]), -o1[1])
            n2 = sh(row(arr, r + o2[0]), -o2[1])
            w |= (cen >= np.maximum(np.maximum(n1, n2), f(MIN2)))
        w50[oi] = w.astype(np.uint8)
        big[oi] = (m_c >= f(MAX2)).astype(np.uint8)
    return w50, big


def _assemble_core(ou, im_pair):
    """ou: (NCH, NPART, 5, RPP, CW) u8 device maps for one core.
    im_pair: (2, 1024, 1024) f32 raw images.
    Returns (e_img, e_week, e_sure) each (2, HO, HO) f32."""
    idx = _pad_idx()
    o = ou.reshape(NCH, NIMG, PPI, 5, RPP, CW)
    o = o.transpose(1, 3, 2, 4, 0, 5).reshape(NIMG, 5, PPI * RPP,
                                              NCH * CW)[:, :, :, :HO]
    e_img = np.empty((NIMG, HO, HO), np.float32)
    e_week = np.empty((NIMG, HO, HO), np.float32)
    e_sure = np.empty((NIMG, HO, HO), np.float32)
    for k in range(NIMG):
        w50 = (o[k, 0] + o[k, 1] + o[k, 2] + o[k, 3])
        sure = w50 * o[k, 4]
        p = im_pair[k][idx][:, idx]
        sw, sb = _strip_rows(p)
        W = np.empty((HO, HO), np.float32)
        S = np.empty((HO, HO), np.float32)
        W[1:1024] = w50[0:1023]
        S[1:1024] = sure[0:1023]
        W[0] = sw[0]
        S[0] = sw[0] * sb[0]
        W[1024] = sw[1]
        S[1024] = sw[1] * sb[1]
        e_img[k] = W * np.float32(255.5)
        e_sure[k] = S * np.float32(255.0)
        e_week[k] = (W - S) * np.float32(255.0)
    return e_img, e_week, e_sure


def kernel(images):
    global LAST_RESULTS
    from concourse.bass_utils import run_bass_kernel_spmd

    images = np.asarray(images, dtype=np.float32)
    assert images.shape == (16, 1024, 1024, 1), images.shape
    im3 = images[:, :, :, 0]
    qcs = _build_qc(im3)

    nc = _get_nc()
    in_maps = [{"qc": qcs[c]} for c in range(8)]
    res = run_bass_kernel_spmd(nc, in_maps, list(range(8)))
    LAST_RESULTS = res

    e_img = np.empty((16, HO, HO, 1), np.float32)
    e_week = np.empty((16, HO, HO, 1), np.float32)
    e_sure = np.empty((16, HO, HO, 1), np.float32)
    for c in range(8):
        ei, ew, es = _assemble_core(res.results[c]["ou"],
                                    im3[c * NIMG:(c + 1) * NIMG])
        e_img[c * NIMG:(c + 1) * NIMG, :, :, 0] = ei
        e_week[c * NIMG:(c + 1) * NIMG, :, :, 0] = ew
        e_sure[c * NIMG:(c + 1) * NIMG, :, :, 0] = es
    return e_img, e_week, e_sure


# revision 51
# speedup vs baseline: 1.2977x; 1.0585x over previous
"""Canny edge detection (nn_CannyEdge_83330955477492) on 8 Trainium2 cores.

Pipeline reproduced from the reference:
  - The reference's "gaussian blur" (sigma=0.05, and a 2x2 kernel thanks to
    arange(-(3//2)+1, 3//2+1) == [0,1]) is exactly a top-left crop of the
    reflect-padded image: blur[i,j] = x[R(i-1), R(j-1)], R(-1)=1 -> 1025x1025.
  - Sobel gx/gy on the reflect-padded blur (correlation).
  - Direction binning by exact slope comparisons on the SQUARES
    (T^2*gx^2 vs gy^2) instead of atan2 (bit-identical except for pixels
    within ~1 ulp of a bin boundary).
  - Magnitude comparisons use gx^2+gy^2 (monotone equivalent of sqrt).
  - NMS via shifted maxes per bin; thresholds at 50^2/100^2.

Sharding: pure data parallel, 2 images per core.

Layout: 128 partitions (64 per image), 16 output rows per partition,
9 column chunks of 116.  The device emits five u8 indicator maps (4 per-bin
weak-edge bits + the >=100 bit) packed in one contiguous store per chunk;
the host sums/crops them into the three 0/255 float maps.  Image rows 0 and
1024 (which see the NMS zero-padding) are recomputed exactly on the host,
which removes all border masking from the device inner loop.

Work is split across three engines, restricted to what the real walrus
backend accepts per engine (Pool = gpsimd: tensor_tensor add/sub/mult and
tensor_scalar only, SBUF only; ACT: 1-input affine+func; DVE: everything):
  Pool: all adds/subs/mults (tt, rsm/gx assembly, dd, t2, gy, sg, mm, the
        masked-magnitude builds) + two bins' weak-bit chains via the exact
        sign-of-subtract trick  w = [cen - max(qt,50^2) >= 0].
  ACT:  the 2*center scaled copies and the gradient squares.
  DVE:  all compares (bin masks, NMS maxes, weak/sure bits).
All arithmetic is plain fp32, bit-identical to the reference up to ulp-level
comparison ties (the fp32r tensor-engine path was rejected: real-HW fp32r
rounds its operands, flipping ~5k near-tie NMS comparisons).
"""
import numpy as np

# ---------------------------------------------------------------- geometry
NIMG = 2             # images per core
H = 1024             # input image size
HO = 1025            # output size (blur is 1025x1025)
RPP = 16             # output rows per partition
PPI = 64             # partitions per image
NPART = NIMG * PPI   # 128
CW = 116             # output cols per chunk
NCH = 9              # chunks (9*116 = 1044 >= 1025)
WA = CW + 4          # loaded cols per chunk (even: fp32r matmul spans)
WB = CW + 2          # stencil cols per chunk (even)
TROWS = 21           # tin rows (20 + 1 pad so shifted flat spans stay in)
QROWS_IMG = 1029     # P's 1027 rows + 2 zero pad
QCOLS = NCH * CW + WA - CW  # 1 zero col + P's 1027 cols + pad
LASTC = HO - CW * (NCH - 1) + 1  # mm col of blur col 1025 in last chunk

_T1 = np.float32(np.tan(np.deg2rad(22.5)))
_T2 = np.float32(np.tan(np.deg2rad(67.5)))
T1S = float(np.float32(_T1 * _T1))
T2S = float(np.float32(_T2 * _T2))
MIN2 = float(np.float32(50.0 * 50.0))
MAX2 = float(np.float32(100.0 * 100.0))

_NC = None
LAST_RESULTS = None  # stashed BassKernelResults for test.py


# ------------------------------------------------- walrus 1-wait workaround
def _split_multiwaits(nc):
    """This walrus build rejects >1 sync wait per instruction: move extra
    waits onto fresh same-engine NOPs inserted right before the carrier."""
    import concourse.mybir as mybir

    n_split = 0
    for fn in nc.m.functions:
        for bb in fn.blocks:
            insts = list(bb.instructions)
            if not any(
                i.sync_info is not None
                and i.sync_info.on_wait
                and len(i.sync_info.on_wait) > 1
                for i in insts
            ):
                continue
            out = []
            for inst in insts:
                si = inst.sync_info
                if si is not None and si.on_wait and len(si.on_wait) > 1:
                    waits = list(si.on_wait)
                    eng = nc.engines[inst.engine]
                    for w in waits[:-1]:
                        nop = eng.nop(hint="waitsplit")
                        # eng.nop() appended to nc.cur_bb — remove it there
                        # (it must live ONLY at its split position, else the
                        # duplicate runs after sem cleanup and deadlocks).
                        host = nc.cur_bb.bb
                        lst = list(host.instructions)
                        assert lst and lst[-1].name == nop.ins.name
                        _set_insts(host, lst[:-1])
                        nop.ins.sync_info = mybir.SyncInfo(
                            on_wait=[w], on_update=[]
                        )
                        out.append(nop.ins)
                        n_split += 1
                    si.on_wait = waits[-1:]
                out.append(inst)
            _set_insts(bb, out)
    return n_split


def _set_insts(bb, lst):
    try:
        bb.instructions = lst
    except Exception:
        bb.instructions.clear()
        bb.instructions.extend(lst)


# ------------------------------------------------------------ device build
#
# v3: the PE (tensor engine) computes every pure add/sub of (shifted) views
# as float32r identity matmuls accumulated in PSUM (bit-exact: products are
# 1.0*x or 2.0*x with exact fp32 PSUM accumulation).  Each matmul output is
# one PSUM bank (3 rows x WB = 393 f32 <= 512); tensors are processed in two
# row-halves so two 4-bank PSUM slots ping-pong.  ACT drains PSUM into dense
# SBUF tiles (fused with Square for gx/gy); DVE/Pool keep the tensor
# multiplies and all NMS compares.
def _build_nc():
    import concourse.bass as bass
    import concourse.tile as tile
    import concourse.mybir as mybir
    from concourse.ap import AP

    f32 = mybir.dt.float32
    f32r = mybir.dt.float32r
    u8 = mybir.dt.uint8
    Alu = mybir.AluOpType
    Act = mybir.ActivationFunctionType

    nc = bass.Bass("TRN2", target_bir_lowering=False, debug=False, num_devices=8)
    qc = nc.declare_dram_parameter("qc", [NCH, NPART, TROWS, WA], f32,
                                   isOutput=False)
    ou = nc.declare_dram_parameter("ou", [NCH, NPART, 5, RPP, CW], u8,
                                   isOutput=True)

    with tile.TileContext(nc) as tc:
        with (
            tc.tile_pool(name="io2", bufs=2) as io2,   # load/store overlap
            tc.tile_pool(name="mid", bufs=1) as mid,   # per-chunk work tiles
        ):
            def stage1(ci):
                """Load + row stencils for chunk ci.  rsm is accumulated in
                place (ACT writes 2*C, Pool adds L then R) so no scratch
                tiles are needed; dd = R - L on Pool."""
                tin = io2.tile([NPART, TROWS, WA], f32, tag="tin")
                src = AP(qc, ci * NPART * TROWS * WA,
                         [[TROWS * WA, NPART], [WA, TROWS], [1, WA]])
                nc.sync.dma_start(out=tin[:], in_=src)

                rsm = mid.tile([NPART, 20, WB], f32, tag="rsm", bufs=2)
                nc.scalar.activation(out=rsm[:], in_=tin[:, 0:20, 1:WB + 1],
                                     func=Act.Copy, scale=2.0)
                nc.gpsimd.tensor_tensor(out=rsm[:], in0=rsm[:],
                                        in1=tin[:, 0:20, 0:WB], op=Alu.add)
                nc.gpsimd.tensor_tensor(out=rsm[:], in0=rsm[:],
                                        in1=tin[:, 0:20, 2:WA], op=Alu.add)
                dd = mid.tile([NPART, 20, WB], f32, tag="dd", bufs=2)
                nc.gpsimd.tensor_tensor(
                    out=dd[:], in0=tin[:, 0:20, 2:WA], in1=tin[:, 0:20, 0:WB],
                    op=Alu.subtract)
                return dict(rsm=rsm, dd=dd)

            def stage2(ci, st):
                """Gradients, sign, squares, bin masks, magnitude."""
                rsm, dd = st["rsm"], st["dd"]
                gx = mid.tile([NPART, 18, WB], f32, tag="gx")
                nc.scalar.activation(out=gx[:], in_=dd[:, 1:19, :],
                                     func=Act.Copy, scale=2.0)
                nc.gpsimd.tensor_tensor(out=gx[:], in0=gx[:],
                                        in1=dd[:, 0:18, :], op=Alu.add)
                nc.gpsimd.tensor_tensor(out=gx[:], in0=gx[:],
                                        in1=dd[:, 2:20, :], op=Alu.add)
                gy = mid.tile([NPART, 18, WB], f32, tag="gy")
                nc.gpsimd.tensor_tensor(
                    out=gy[:], in0=rsm[:, 2:20, :], in1=rsm[:, 0:18, :],
                    op=Alu.subtract)
                sg = mid.tile([NPART, 18, WB], f32, tag="sg", bufs=2)
                nc.gpsimd.tensor_tensor(out=sg[:], in0=gx[:], in1=gy[:],
                                        op=Alu.mult)
                nc.scalar.activation(out=sg[:], in_=sg[:], func=Act.Sign)
                # squares in place (ACT)
                nc.scalar.activation(out=gx[:], in_=gx[:], func=Act.Square)
                nc.scalar.activation(out=gy[:], in_=gy[:], func=Act.Square)

                # bin masks from the squares (DVE)
                # c0 holds sign(T1S*gx^2 - gy^2): +1 -> bin0, -1 -> diag
                c0 = mid.tile([NPART, 18, WB], f32, tag="c0", bufs=2)
                nc.vector.scalar_tensor_tensor(
                    out=c0[:], in0=gx[:], scalar=T1S, in1=gy[:],
                    op0=Alu.mult, op1=Alu.subtract)
                nc.scalar.activation(out=c0[:], in_=c0[:], func=Act.Sign)
                # d2 holds sign(T2S*gx^2 - gy^2): +1 -> steep (vert off)
                d2 = mid.tile([NPART, 18, WB], f32, tag="d2", bufs=2)
                nc.vector.scalar_tensor_tensor(
                    out=d2[:], in0=gx[:], scalar=T2S, in1=gy[:],
                    op0=Alu.mult, op1=Alu.subtract)
                nc.scalar.activation(out=d2[:], in_=d2[:], func=Act.Sign)

                # magnitude^2 + NMS zero-pad cols at image edges
                mm = mid.tile([NPART, 18, WB], f32, tag="mm", bufs=2)
                nc.gpsimd.tensor_tensor(out=mm[:], in0=gx[:], in1=gy[:],
                                        op=Alu.add)
                if ci == 0:
                    nc.gpsimd.memset(mm[:, :, 0:1], 0.0)       # blur col -1
                if ci == NCH - 1:
                    nc.gpsimd.memset(mm[:, :, LASTC:LASTC + 1], 0.0)
                return dict(sg=sg, c0=c0, d2=d2, mm=mm)

            def stage3(ci, st):
                """NMS per-bin processing + store for chunk ci."""
                sg, c0, d2, mm = st["sg"], st["c0"], st["d2"], st["mm"]
                ws = io2.tile([NPART, 5, RPP, CW], u8, tag="ws")

                def wslot(s):
                    return ws[:, s:s + 1].rearrange("p a r c -> p (a r) c")

                nc.vector.tensor_scalar(
                    out=wslot(4), in0=mm[:, 1:17, 1:1 + CW], scalar1=MAX2,
                    scalar2=None, op0=Alu.is_ge)

                def nms_bin(ang, r1, c1, r2, c2, slot):
                    qt = mid.tile([NPART, RPP, CW], f32, tag="qt", bufs=2)
                    nc.vector.tensor_tensor(
                        out=qt[:],
                        in0=ang[:, r1:r1 + RPP, c1:c1 + CW],
                        in1=ang[:, r2:r2 + RPP, c2:c2 + CW],
                        op=Alu.max)
                    nc.vector.scalar_tensor_tensor(
                        out=wslot(slot), in0=qt[:], scalar=MIN2,
                        in1=ang[:, 1:17, 1:1 + CW], op0=Alu.max,
                        op1=Alu.is_le)

                # bins are disjoint; each w bit is 1 only where the center
                # is in the bin and >= max(neighbors, 50^2).
                # smm = M*sign(e2): steep pixels +M, flat -M.  Then
                # ang2 = Relu(-smm) and Md2 = Relu(smm) both on ACT.
                md2 = mid.tile([NPART, 18, WB], f32, tag="md2")
                nc.gpsimd.tensor_tensor(out=md2[:], in0=mm[:], in1=d2[:],
                                        op=Alu.mult)
                angA = mid.tile([NPART, 18, WB], f32, tag="angA")
                nc.scalar.activation(out=angA[:], in_=md2[:], func=Act.Relu,
                                     scale=-1.0)          # ang2 = M*(1-d2)
                nc.scalar.activation(out=md2[:], in_=md2[:], func=Act.Relu)
                nms_bin(angA, 0, 1, 2, 1, 2)              # bin2: up/down
                # signed horizontal: sa = Md2*sign(e0); bin0 = sa>0 side,
                # the diagonals live on sa<0 with magnitude mdiag = Relu(-sa)
                angB = mid.tile([NPART, 18, WB], f32, tag="angB")
                nc.gpsimd.tensor_tensor(out=angB[:], in0=md2[:], in1=c0[:],
                                        op=Alu.mult)
                nms_bin(angB, 1, 0, 1, 2, 0)              # bin0: left/right
                mdg = mid.tile([NPART, 18, WB], f32, tag="angA")
                nc.scalar.activation(out=mdg[:], in_=angB[:], func=Act.Relu,
                                     scale=-1.0)          # mdiag
                # signed diagonal magnitude: smd = sign(gx*gy) * mdiag.
                # bin3 = smd>0 pixels, bin1 = smd<0; one array serves both:
                #   w3 = [smd_c >= max(smd_n1, smd_n2,  MIN2)]  (main diag)
                #   w1 = [smd_c <= min(smd_m1, smd_m2, -MIN2)]  (anti diag)
                # (sg==0 implies mdiag==0, so the sign-0 case is inert.)
                smd = mid.tile([NPART, 18, WB], f32, tag="angC")
                nc.gpsimd.tensor_tensor(out=smd[:], in0=sg[:], in1=mdg[:],
                                        op=Alu.mult)
                nms_bin(smd, 0, 0, 2, 2, 3)               # bin3: main diag
                qt1 = mid.tile([NPART, RPP, CW], f32, tag="qt", bufs=2)
                nc.vector.tensor_tensor(
                    out=qt1[:], in0=smd[:, 0:RPP, 2:2 + CW],
                    in1=smd[:, 2:2 + RPP, 0:CW], op=Alu.min)
                nc.vector.scalar_tensor_tensor(
                    out=wslot(1), in0=qt1[:], scalar=-MIN2,
                    in1=smd[:, 1:17, 1:1 + CW], op0=Alu.min, op1=Alu.is_ge)

                dst = AP(ou, ci * NPART * 5 * RPP * CW,
                         [[5 * RPP * CW, NPART], [RPP * CW, 5],
                          [CW, RPP], [1, CW]])
                nc.sync.dma_start(out=dst, in_=ws[:])

            # 3-deep software pipeline
            s1 = {}
            s2 = {}
            for ci in range(NCH + 2):
                if 1 <= ci <= NCH:
                    s2[ci - 1] = stage2(ci - 1, s1.pop(ci - 1))
                if ci >= 2:
                    stage3(ci - 2, s2.pop(ci - 2))
                if ci < NCH:
                    s1[ci] = stage1(ci)

    _split_multiwaits(nc)
    return nc


def _get_nc():
    global _NC
    if _NC is None:
        _NC = _build_nc()
    return _NC


# ------------------------------------------------------------- host helpers
def _eye3():
    """Stationary matrices for the PE identity matmuls: I, 2I, -I."""
    e = np.eye(NPART, dtype=np.float32)
    return np.stack([e, 2.0 * e, -e], axis=1).astype(np.float32)


def _pad_idx():
    idx = np.empty(1027, np.int64)
    idx[0] = 0
    idx[1] = 1
    idx[2:1026] = np.arange(1024)
    idx[1026] = 1022
    return idx


def _build_qc(images):
    """images: (16, 1024, 1024) f32 -> per-core chunked input
    (8, NCH, NPART, 20, WA) with all halos baked in so every load is one
    contiguous descriptor per partition."""
    idx = _pad_idx()
    qcs = np.zeros((8, NCH, NPART, TROWS, WA), np.float32)
    win = np.zeros((NPART, TROWS, QCOLS), np.float32)
    for core in range(8):
        for k in range(NIMG):
            im = images[core * NIMG + k]
            p = im[idx][:, idx]                      # (1027, 1027) = bp rows
            q = np.zeros((QROWS_IMG, QCOLS), np.float32)
            q[0:1027, 1:1028] = p
            # partition p' takes rows 16p'..16p'+20
            sw = np.lib.stride_tricks.sliding_window_view(q, TROWS, axis=0)
            win[k * PPI:(k + 1) * PPI] = sw[0:16 * PPI:16].transpose(0, 2, 1)
        for ci in range(NCH):
            qcs[core, ci] = win[:, :, CW * ci:CW * ci + WA]
    return qcs


def _strip_rows(p):
    """Exact f32 Canny decision bits for output rows 0 and 1024.

    p: (1027, 1027) f32 padded blur (rows/cols -1..1025).
    Returns (w50, big) as uint8 arrays of shape (2, 1025)."""
    f = np.float32
    w50 = np.zeros((2, HO), np.uint8)
    big = np.zeros((2, HO), np.uint8)
    for oi, r in enumerate((0, H)):
        rows = [j for j in (r - 1, r, r + 1) if 0 <= j <= H]
        mm = {}
        a2 = {}
        a0 = {}
        a3 = {}
        a1 = {}
        for j in rows:
            s = p[j:j + 3]                           # 3 x 1027
            ddr = s[:, 2:] - s[:, :-2]               # 3 x 1025
            ssr = s[:, :-2] + f(2.0) * s[:, 1:-1] + s[:, 2:]
            gx = ddr[0] + f(2.0) * ddr[1] + ddr[2]
            gy = ssr[2] - ssr[0]
            m = gx * gx + gy * gy
            ax = np.abs(gx)
            ay = np.abs(gy)
            c0 = (_T1 * ax >= ay)
            d2 = (_T2 * ax > ay)
            sgp = (gx * gy) > 0
            mm[j] = m
            a2[j] = np.where(d2, f(0), m)
            md = np.where(d2, m, f(0))
            a0[j] = np.where(c0, md, f(0))
            mdg = md - a0[j]
            a3[j] = np.where(sgp, mdg, f(0))
            a1[j] = mdg - a3[j]
        z = np.zeros(HO, np.float32)

        def sh(v, d):  # shift cols by d with zero pad
            if d == 0:
                return v
            o = np.zeros_like(v)
            if d > 0:
                o[d:] = v[:-d]
            else:
                o[:d] = v[-d:]
            return o

        def row(arr, j):
            return arr[j] if j in arr else z

        m_c = mm[r]
        w = np.zeros(HO, bool)
        for arr, (o1, o2) in ((a2, ((-1, 0), (1, 0))),
                              (a0, ((0, -1), (0, 1))),
                              (a3, ((-1, -1), (1, 1))),
                              (a1, ((-1, 1), (1, -1)))):
            cen = row(arr, r)
            # sh(v, d) yields o[c] = v[c - d]; neighbor at col c + dc
            # therefore needs d = -dc.
            n1 = sh(row(arr, r + o1[0]), -o1[1])
            n2 = sh(row(arr, r + o2[0]), -o2[1])
            w |= (cen >= np.maximum(np.maximum(n1, n2), f(MIN2)))
        w50[oi] = w.astype(np.uint8)
        big[oi] = (m_c >= f(MAX2)).astype(np.uint8)
    return w50, big


def _assemble_core(ou, im_pair):
    """ou: (NCH, NPART, 5, RPP, CW) u8 device maps for one core.
    im_pair: (2, 1024, 1024) f32 raw images.
    Returns (e_img, e_week, e_sure) each (2, HO, HO) f32."""
    idx = _pad_idx()
    o = ou.reshape(NCH, NIMG, PPI, 5, RPP, CW)
    o = o.transpose(1, 3, 2, 4, 0, 5).reshape(NIMG, 5, PPI * RPP,
                                              NCH * CW)[:, :, :, :HO]
    e_img = np.empty((NIMG, HO, HO), np.float32)
    e_week = np.empty((NIMG, HO, HO), np.float32)
    e_sure = np.empty((NIMG, HO, HO), np.float32)
    for k in range(NIMG):
        w50 = (o[k, 0] + o[k, 1] + o[k, 2] + o[k, 3])
        sure = w50 * o[k, 4]
        p = im_pair[k][idx][:, idx]
        sw, sb = _strip_rows(p)
        W = np.empty((HO, HO), np.float32)
        S = np.empty((HO, HO), np.float32)
        W[1:1024] = w50[0:1023]
        S[1:1024] = sure[0:1023]
        W[0] = sw[0]
        S[0] = sw[0] * sb[0]
        W[1024] = sw[1]
        S[1024] = sw[1] * sb[1]
        e_img[k] = W * np.float32(255.5)
        e_sure[k] = S * np.float32(255.0)
        e_week[k] = (W - S) * np.float32(255.0)
    return e_img, e_week, e_sure


def kernel(images):
    global LAST_RESULTS
    from concourse.bass_utils import run_bass_kernel_spmd

    images = np.asarray(images, dtype=np.float32)
    assert images.shape == (16, 1024, 1024, 1), images.shape
    im3 = images[:, :, :, 0]
    qcs = _build_qc(im3)

    nc = _get_nc()
    in_maps = [{"qc": qcs[c]} for c in range(8)]
    res = run_bass_kernel_spmd(nc, in_maps, list(range(8)))
    LAST_RESULTS = res

    e_img = np.empty((16, HO, HO, 1), np.float32)
    e_week = np.empty((16, HO, HO, 1), np.float32)
    e_sure = np.empty((16, HO, HO, 1), np.float32)
    for c in range(8):
        ei, ew, es = _assemble_core(res.results[c]["ou"],
                                    im3[c * NIMG:(c + 1) * NIMG])
        e_img[c * NIMG:(c + 1) * NIMG, :, :, 0] = ei
        e_week[c * NIMG:(c + 1) * NIMG, :, :, 0] = ew
        e_sure[c * NIMG:(c + 1) * NIMG, :, :, 0] = es
    return e_img, e_week, e_sure
